# revision 25
# baseline (speedup 1.0000x reference)
"""Bilinear LTN scoring kernel for Trainium2 (8 NeuronCores).

scores[i] = h_emb[h[i]]^T @ W[r[i]] @ t_emb[t[i]],  B=4096, DIM=256.

Strategy: the batch shares only N_REL=500 relation matrices (256KB each),
so items are grouped by relation and the *relations* are sharded across
the 8 cores — each relation matrix streams from HBM exactly once
system-wide (~10MB/core) instead of once per item (131MB/core).

Layout (per core, all static):
  - relation chunks of <=C=32 items = "groups"; a matmul computes one
    group's 32 scores against its W (stationary 32 cols, moving W 256)
  - groups pack into "superblocks": 16 group slots = 512 grid slots = 128
    compact items (chunks split across superblocks as needed; W row
    duplicated). SB superblocks per core.
  - h/t entity rows are gathered *compactly* (128 rows per indirect DMA,
    ~5 calls instead of 16+) — indirect-DMA descriptor emission on the
    GpSimd Q7 is ~1.2us per 128 rows and would otherwise serialize.
  - compact -> grid expansion is free for H: the PE transpose that builds
    the [dim, slot] stationary layout uses a one-hot permutation matrix P
    instead of the identity (out = in^T @ P). For T (bf16 mode) the same
    P expands T via one matmul per grid block.
  - pad slots map to zero rows of P, so pads cost nothing and produce 0.
  - ~40 dummy matmuls at kernel start warm the PE HAM clock gate to
    2.4GHz before the real stream arrives.

Precision modes (MODE):
  bf16   — all matmul operands bf16 (~2e-3 scale-relative error)
  bf16x2 — W and H split into bf16 hi+lo, 3 cross terms accumulated in
           PSUM, T kept fp32: ~4e-6 scale-relative error
"""

import sys

for _p in ("/opt/trn_rl_repo",):
    if _p not in sys.path:
        sys.path.insert(0, _p)

import ml_dtypes
import numpy as np

import concourse.bass as bass
import concourse.mybir as mybir
import concourse.tile as tile
from concourse.bass import IndirectOffsetOnAxis
from concourse.bass_utils import run_bass_kernel_spmd
from concourse.vector_clock import ScopedClock

DIM = 256
N_ENT = 100000
N_REL = 500
NCORES = 8
C = 32                 # items per group (matmul stationary width)
SB_GROUPS = 16         # group slots per superblock
SB_ITEMS = 128         # compact items per superblock (= 4 grid blocks)
PAD_IDX = 0x7FFF0000   # > N_ENT-1 -> indirect DMA skips the row

F32 = mybir.dt.float32
BF16 = mybir.dt.bfloat16
FP16 = mybir.dt.float16
I32 = mybir.dt.int32

MODE = "fp16"  # "fp16" | "bf16" | "bf16x2"

_MAX_WAITS = 1


def _install_walrus_fixes():
    """This container's walrus accepts only one sync wait per instruction;
    split extra waits onto preceding same-engine NOPs."""
    if getattr(tile.TileContext, "_drain_fix_installed", False):
        return

    def _split_multi_waits(nc):
        cur_bb = nc.cur_bb.bb
        for f in nc.m.functions:
            for blk in f.blocks:
                bb = blk if hasattr(blk, "instructions") else blk.bb
                i = 0
                while i < len(bb.instructions):
                    inst = bb.instructions[i]
                    si = getattr(inst, "sync_info", None)
                    waits = list(si.on_wait or []) if si is not None else []
                    if len(waits) > _MAX_WAITS:
                        si.on_wait = waits[-_MAX_WAITS:]
                        extra = waits[: -_MAX_WAITS]
                        nops = []
                        for w0 in range(0, len(extra), _MAX_WAITS):
                            nop_inst = nc.engines[inst.engine].nop(
                                nofuse=True, hint="wait_split"
                            )
                            nop_inst.ins.sync_info = mybir.SyncInfo(
                                on_wait=extra[w0 : w0 + _MAX_WAITS],
                                on_update=[],
                            )
                            nops.append(nop_inst.ins)
                        for n in nops:
                            cur_bb.instructions.remove(n)
                        for j, n in enumerate(nops):
                            bb.instructions.insert(i + j, n)
                        i += len(nops)
                    i += 1

    def _drain_and_barrier(self, tick_clock, wait_clock):
        drain_inst = self.nc.sync.drain()
        wait_clock.add_sem_waits(
            drain_inst.ins, ScopedClock({None: tick_clock.global_clock})
        )
        self.nc.all_engine_barrier()
        assert self.sems is not None
        popped = self.nc._tile_sem_poison_stack.pop()
        assert popped is self._sem_poison
        self.nc.clear_and_free_semaphores(list(self.sems.allocated().values()))
        self.nc.all_engine_barrier()
        _split_multi_waits(self.nc)

    tile.TileContext._drain_and_barrier = _drain_and_barrier
    tile.TileContext._drain_fix_installed = True


def _build_bass(mode, SB):
    _install_walrus_fixes()
    x2 = mode == "bf16x2"
    MM = FP16 if mode == "fp16" else BF16
    nw = 2 if x2 else 1          # W streams (hi / hi+lo)
    G = SB * SB_GROUPS           # group slots per core
    NCB = SB                     # compact blocks (128 rows each)
    GRID_BLOCKS = SB * 4         # [128, 256] grid blocks
    GRID_COLS = GRID_BLOCKS      # out is [128, GRID_BLOCKS]
    NPAIR = SB * 4               # (superblock, q) permutation blocks

    nc = bass.Bass()
    ent = nc.declare_dram_parameter("ent", [N_ENT, DIM], F32, isOutput=False)
    wrows = nc.declare_dram_parameter(
        "wrows", [nw * G, DIM * DIM], MM, isOutput=False
    )
    hidx = nc.declare_dram_parameter("hidx", [128, NCB], I32, isOutput=False)
    t_cols = GRID_BLOCKS
    tidx = nc.declare_dram_parameter("tidx", [128, t_cols], I32, isOutput=False)
    pmat = nc.declare_dram_parameter(
        "pmat", [NPAIR, 128 * 128], MM, isOutput=False
    )
    out = nc.declare_dram_parameter("out", [128, GRID_COLS], F32, isOutput=True)

    with tile.TileContext(nc) as tc:
        with (
            tc.tile_pool(name="const", bufs=1) as const_pool,
            tc.tile_pool(name="gather", bufs=1) as gather_pool,
            tc.tile_pool(name="ht", bufs=1) as ht_pool,
            tc.tile_pool(name="w", bufs=10) as w_pool,
            tc.tile_pool(name="scratch", bufs=2) as scratch_pool,
            tc.tile_pool(name="upsum", bufs=(5 if x2 else 7), space="PSUM") as u_pool,
            tc.tile_pool(name="trpsum", bufs=1, space="PSUM") as tr_pool,
        ):
            # ---- PE warm-up: dense dummy matmuls, no data deps
            dummy = const_pool.tile([128, DIM], MM, tag="dummy")
            nc.vector.memset(dummy[:], 0.0)
            dps = u_pool.tile([128, DIM], F32, space="PSUM", tag="ups", name="dps")
            for wu in range(24):
                nc.tensor.matmul(
                    out=dps[0:32, :],
                    lhsT=dummy[:, 0:32],
                    rhs=dummy[:],
                    start=True,
                    stop=True,
                    tile_position=(0, 0),
                )

            hidx_t = const_pool.tile([128, NCB], I32, tag="hidx")
            tidx_t = const_pool.tile([128, t_cols], I32, tag="tidx")
            tidx_grid_t = tidx_t
            nc.sync.dma_start(out=hidx_t[:], in_=hidx[:])
            nc.sync.dma_start(out=tidx_t[:], in_=tidx[:])

            # permutation blocks: pt[:, i*128:(i+1)*128] = P_i [c, g]
            pt = const_pool.tile([128, NPAIR * 128], MM, tag="pt")
            nc.sync.dma_start(
                out=pt[:].rearrange("c (i g) -> c i g", i=NPAIR, g=128),
                in_=pmat[:].rearrange("i (c g) -> c i g", c=128, g=128),
            )

            out_sb = const_pool.tile([128, GRID_COLS], F32, tag="outsb")

            # ---- compact gathers (h first; t after)
            hc = [
                gather_pool.tile([128, DIM], F32, tag=f"hc{b}", name=f"hc{b}")
                for b in range(NCB)
            ]
            for b in range(NCB):
                nc.vector.memset(hc[b][:], 0.0)

            def _h_gather(b):
                nc.gpsimd.indirect_dma_start(
                    out=hc[b][:],
                    out_offset=None,
                    in_=ent[:],
                    in_offset=IndirectOffsetOnAxis(ap=hidx_t[:, b : b + 1], axis=0),
                    bounds_check=N_ENT - 1,
                    oob_is_err=False,
                )
            for b in range(min(2, NCB)):
                _h_gather(b)
            # T gathered directly in grid layout (stays fp32)
            tg = [
                gather_pool.tile([128, DIM], F32, tag=f"tg{j}", name=f"tg{j}")
                for j in range(GRID_BLOCKS)
            ]
            for b in range(2, NCB):
                _h_gather(b)
            for j in range(GRID_BLOCKS):
                nc.gpsimd.indirect_dma_start(
                    out=tg[j][:],
                    out_offset=None,
                    in_=ent[:],
                    in_offset=IndirectOffsetOnAxis(
                        ap=tidx_t[:, j : j + 1], axis=0
                    ),
                    bounds_check=N_ENT - 1,
                    oob_is_err=False,
                )

            # ---- casts to bf16 (+ hi/lo residual for x2)
            hcb = []
            hlob = []
            for b in range(NCB):
                hb = gather_pool.tile([128, DIM], MM, tag=f"hcb{b}", name=f"hcb{b}")
                nc.vector.tensor_copy(hb[:], hc[b][:])
                hcb.append(hb)
                if x2:
                    lo32 = scratch_pool.tile(
                        [128, DIM], F32, tag="lo32", name=f"lo32_{b}"
                    )
                    nc.vector.tensor_tensor(
                        out=lo32[:],
                        in0=hc[b][:],
                        in1=hb[:],
                        op=mybir.AluOpType.subtract,
                    )
                    lb = gather_pool.tile(
                        [128, DIM], MM, tag=f"hlob{b}", name=f"hlob{b}"
                    )
                    nc.scalar.copy(lb[:], lo32[:])
                    hlob.append(lb)
            # ---- permuting transposes: ht[k][j][a, g] = H[grid j*128+g][128k+a]
            ht = [[None] * GRID_BLOCKS for _ in range(2)]
            htlo = [[None] * GRID_BLOCKS for _ in range(2)] if x2 else None
            for sb in range(SB):
                for q in range(4):
                    j = 4 * sb + q
                    psl = pt[:, j * 128 : (j + 1) * 128]
                    for k in range(2):
                        trp = tr_pool.tile([128, 128], MM, space="PSUM", tag="trp", name=f"trp{j}_{k}")
                        nc.tensor.matmul(
                            out=trp[:],
                            lhsT=hcb[sb][:, k * 128 : (k + 1) * 128],
                            rhs=psl,
                            is_transpose=True,
                        )
                        htt = ht_pool.tile(
                            [128, 128], MM, tag=f"ht{k}_{j}", name=f"ht{k}_{j}"
                        )
                        ht[k][j] = htt
                        if (j * 2 + k) % 2 == 0:
                            nc.vector.tensor_copy(htt[:], trp[:])
                        else:
                            nc.scalar.copy(htt[:], trp[:])
                        if x2:
                            trpl = tr_pool.tile(
                                [128, 128], MM, space="PSUM", tag="trpl",
                                name=f"trpl{j}_{k}"
                            )
                            nc.tensor.matmul(
                                out=trpl[:],
                                lhsT=hlob[sb][:, k * 128 : (k + 1) * 128],
                                rhs=psl,
                                is_transpose=True,
                            )
                            htl = ht_pool.tile(
                                [128, 128], MM, tag=f"htlo{k}_{j}", name=f"htlo{k}_{j}"
                            )
                            htlo[k][j] = htl
                            if (j * 2 + k) % 2 == 0:
                                nc.scalar.copy(htl[:], trpl[:])
                            else:
                                nc.vector.tensor_copy(htl[:], trpl[:])

            # ---- relation-matrix stream + per-group matmuls + reduce
            for m in range(G // 4):  # 4 groups (= 1 grid block) per DMA
                wts = []
                for s in range(nw):
                    wt = w_pool.tile(
                        [128, 2048], MM, tag=f"wt{s}", name=f"wt{s}_{m}"
                    )
                    dma_eng = nc.sync if (m + s) % 2 == 0 else nc.scalar
                    dma_eng.dma_start(
                        out=wt[:].rearrange("p (g k b) -> p g k b", g=4, k=2, b=DIM),
                        in_=wrows[
                            s * G + 4 * m : s * G + 4 * m + 4
                        ].rearrange("g (k a b) -> a g k b", k=2, a=128, b=DIM),
                    )
                    wts.append(wt)
                ups = u_pool.tile([128, DIM], F32, space="PSUM", tag="ups", name=f"ups{m}")
                j = m
                if x2:
                    terms = [
                        (ht[0], 0, 0),
                        (ht[1], 0, 1),
                        (ht[0], 1, 0),
                        (ht[1], 1, 1),
                        (htlo[0], 0, 0),
                        (htlo[1], 0, 1),
                    ]
                else:
                    terms = [(ht[0], 0, 0), (ht[1], 0, 1)]
                nt = len(terms)
                # term-outer / group-inner: the 4 groups hit disjoint 32-col
                # PSUM strips, so consecutive matmuls col-tile concurrently
                for ti, (htk, s, k) in enumerate(terms):
                    for d in range(4):
                        part = 32 * d
                        nc.tensor.matmul(
                            out=ups[part : part + 32, :],
                            lhsT=htk[j][:, part : part + 32],
                            rhs=wts[s][:, (d * 2 + k) * DIM : (d * 2 + k + 1) * DIM],
                            start=(ti == 0),
                            stop=(ti == nt - 1),
                            tile_position=(0, part),
                        )
                sc = scratch_pool.tile([128, DIM], F32, tag="sc", name=f"sc{m}")
                nc.vector.tensor_tensor(
                    out=sc[:],
                    in0=ups[:],
                    in1=tg[j][:],
                    op=mybir.AluOpType.mult,
                )
                nc.vector.tensor_reduce(
                    out=out_sb[:, j : j + 1],
                    in_=sc[:],
                    axis=mybir.AxisListType.X,
                    op=mybir.AluOpType.add,
                )

            nc.sync.dma_start(out=out[:], in_=out_sb[:])

    return nc


_NC_CACHE = {}


def _get_nc(mode, SB):
    key = (mode, SB)
    if key not in _NC_CACHE:
        _NC_CACHE[key] = _build_bass(mode, SB)
    return _NC_CACHE[key]


def _pack(h, r, t, rel_weight, mode):
    """Group items by relation, chunk to <=C, balance chunks across cores,
    pack into superblocks of <=SB_GROUPS groups / <=SB_ITEMS items."""
    x2 = mode == "bf16x2"
    order = np.argsort(r, kind="stable")
    rs = r[order]
    starts = np.flatnonzero(np.r_[True, rs[1:] != rs[:-1]])
    ends = np.r_[starts[1:], len(rs)]
    chunks = []  # (rel_id, item_positions)
    for s0, e0 in zip(starts, ends):
        rid = int(rs[s0])
        for c0 in range(s0, e0, C):
            chunks.append((rid, order[c0 : min(c0 + C, e0)]))
    chunks.sort(key=lambda x: -len(x[1]))

    per_core = [[] for _ in range(NCORES)]
    counts = [0] * NCORES
    items = [0] * NCORES
    for ch in chunks:
        k = min(range(NCORES), key=lambda q: (counts[q], items[q]))
        per_core[k].append(ch)
        counts[k] += 1
        items[k] += len(ch[1])

    # pack each core's chunks into superblocks
    core_sbs = []
    for k in range(NCORES):
        sbs = [[]]  # list of superblocks; each = list of (rid, positions)
        sb_items = 0
        for rid, pos in per_core[k]:
            pos = np.asarray(pos)
            while len(pos):
                if len(sbs[-1]) >= SB_GROUPS or sb_items >= SB_ITEMS:
                    sbs.append([])
                    sb_items = 0
                take = min(len(pos), SB_ITEMS - sb_items)
                sbs[-1].append((rid, pos[:take]))
                sb_items += take
                pos = pos[take:]
        core_sbs.append(sbs)
    SB = max(max(len(s) for s in core_sbs), 2)

    nw = 2 if x2 else 1
    G = SB * SB_GROUPS
    NCB = SB
    GRID_BLOCKS = SB * 4
    t_cols = GRID_BLOCKS

    bf = np.float16 if mode == "fp16" else ml_dtypes.bfloat16
    in_maps = []
    slot_maps = []
    for k in range(NCORES):
        wrows = np.zeros((nw * G, DIM * DIM), dtype=bf)
        hidx = np.full((128, NCB), PAD_IDX, dtype=np.int32)
        tidx = np.full((128, t_cols), PAD_IDX, dtype=np.int32)
        pmat = np.zeros((GRID_BLOCKS, 128 * 128), dtype=bf)
        pmat3 = pmat.reshape(GRID_BLOCKS, 128, 128)
        slots = []
        positions = []
        for sb, sblist in enumerate(core_sbs[k]):
            cpos = 0  # compact position within superblock
            for g_local, (rid, pos) in enumerate(sblist):
                gslot = sb * SB_GROUPS + g_local
                w32 = rel_weight[rid]
                whi = w32.astype(bf)
                wrows[gslot] = whi
                if nw == 2:
                    wrows[G + gslot] = (w32 - whi.astype(np.float32)).astype(bf)
                s = gslot * C + np.arange(len(pos))       # grid slots
                c = sb * SB_ITEMS + cpos + np.arange(len(pos))  # compact slots
                cpos += len(pos)
                hidx[c % 128, c // 128] = h[pos]
                tidx[s % 128, s // 128] = t[pos]
                # P block (4sb+q): [c%128, s%128] for items in grid block
                pmat3[s // 128, c % 128, s % 128] = 1
                slots.append(s)
                positions.append(pos)
        slots = np.concatenate(slots) if slots else np.zeros(0, np.int64)
        positions = (
            np.concatenate(positions) if positions else np.zeros(0, np.int64)
        )
        slot_maps.append((slots, positions))
        in_maps.append(
            {"wrows": wrows, "hidx": hidx, "tidx": tidx, "pmat": pmat}
        )
    return in_maps, slot_maps, SB


def _run(h, r, t, ent_weight, rel_weight, trace=False, mode=None):
    if mode is None:
        mode = MODE
    h = np.asarray(h).astype(np.int64)
    r = np.asarray(r).astype(np.int64)
    t = np.asarray(t).astype(np.int64)
    ent_weight = np.ascontiguousarray(np.asarray(ent_weight, dtype=np.float32))
    rel_weight = np.ascontiguousarray(np.asarray(rel_weight, dtype=np.float32))
    assert ent_weight.shape == (N_ENT, DIM)
    assert rel_weight.shape == (N_REL, DIM * DIM)

    in_maps, slot_maps, SB = _pack(h, r, t, rel_weight, mode)
    for im in in_maps:
        im["ent"] = ent_weight
    nc = _get_nc(mode, SB)
    res = run_bass_kernel_spmd(
        nc, in_maps, core_ids=list(range(NCORES)), trace=trace
    )
    scores = np.empty(h.shape[0], dtype=np.float32)
    for k in range(NCORES):
        o = res.results[k]["out"]
        slots, positions = slot_maps[k]
        scores[positions] = o[slots % 128, slots // 128]
    return scores, res


def kernel(h, r, t, ent_weight, rel_weight):
    scores, _ = _run(h, r, t, ent_weight, rel_weight, trace=False)
    return scores
